# revision 29
# baseline (speedup 1.0000x reference)
"""Group-quantized linear (fake int4 per-group dequant) GEMV on 8 Trainium2 cores.

Reference computation (all fp32):
    qw = round_half_even(clip(W, -8, 7))            # W in [-8, 7) so clip is identity
    out = (qw.reshape(O, 64, 128) * scales[:, :, None]).reshape(O, O) @ x

Sharding: column-parallel — each core owns a 1024-row slice of W/scales,
x replicated, outputs concatenated (per the tensor-parallel hint).  All
host-side work is pure layout: the per-core weight slice ships fully
permuted to the SBUF tile order [c(128), ch(8), gp(8), o(1024)], x ships
pre-transposed [128, 64], scales ship pre-arranged (and duplicated over
the hi/lo axis), and the [128, 8] result is un-permuted on the host — no
on-device transposes at all.

DMA-engine load balancing (all measured on this part):
  - A full-tile [0:128] DMA with 16 KiB descriptors streams at ~420 GB/s
    aggregate: each of the 16 DMA engines E64..E79 takes the 8 descriptors
    of its partition group at ~26.9 GB/s — EXCEPT E79, which only manages
    ~21.6 GB/s, so an all-uniform stream is E79-bound at ~97 us.
  - A [0:120]-partition DMA dispatches over E64..E78 ONLY (E79-free), but
    that mode runs its descriptors at half rate (~13 GB/s).
  - Unaligned partition counts (17, 127) collapse onto a single engine;
    concurrent big streams on two rings also degrade — avoid both.
Optimum: ship most bytes uniform, and divert ~3 MiB via E79-free quarter
DMAs so every engine finishes together:
  13 "uniform" half-chunks: one [0:128] DMA, 16 KiB descs
   3 "mixed"  half-chunks: U [0:128] gp 0:2 (8 KiB descs)
                         + F [0:120] gp 2:4 (8 KiB descs, E79-free)
                         + R [120:128] gp 2:4 (4 KiB descs, uniform)
Per-engine busy: E79 ~87 us, E64..E78 ~86 us — balanced, no straggler.
x / scales ride the scalar-engine ring (tiny); the last half-chunk ships
as four per-group DMAs so the post-stream quantize tail is ~0.7 us.

Per-core pipeline (device):
  DVE   : quantize via the fp32 magic-number trick (w + 1.5*2^23) - 1.5*2^23
          == round-half-even exactly for |w| < 2^22, cast to bf16 (exact for
          ints in [-8, 7]); one tensor_scalar per half-chunk
  PE    : per (group g, out-chunk oc) matmul acc[:, oc, gp, :2] =
          qwT[128c, 128o].T @ x2[128c, 2] where x2 = [x_hi | x_lo] bf16
          Dekker split of x (fp32-accurate), fp32 PSUM; acc is one ping-pong
          PSUM bank per chunk
  DVE   : per chunk, THREE ops covering all out-chunks at once:
          y[128, oc, gp, 2] = acc * sc2[:, ch]      (PSUM read, one TT)
          partial[128, oc]  = reduce_sum(y, XY)     (hi/lo + group reduction)
          out_acc           = out_acc + partial     (ping-pong buffers)

HBM traffic/core = 32 MiB weights => ~87 us balanced stream.
"""

import numpy as np

IN_DIM = 8192
OUT_DIM = 8192
NUM_GROUPS = 64
GROUP_SIZE = 128  # IN_DIM // NUM_GROUPS
N_CORES = 8
PER_OUT = OUT_DIM // N_CORES  # 1024
P = 128

MAGIC = np.float32(12582912.0)  # 1.5 * 2**23: (w + MAGIC) - MAGIC == rint(w)

_cache = {}


def _split_multi_waits(nc):
    """walrus in this container accepts only ONE sync-wait per instruction;
    Tile's tail drain carries one per producer proc. Hoist extras onto
    same-engine NoOps placed immediately before — identical semantics for an
    in-order sequencer."""
    import concourse.mybir as mybir

    uid = 0
    for f in nc.m.functions:
        for blk in f.blocks:
            insts = blk.instructions
            if not any(
                i.sync_info is not None
                and i.sync_info.on_wait
                and len(i.sync_info.on_wait) > 1
                for i in insts
            ):
                continue
            new_insts = []
            for inst in insts:
                si = inst.sync_info
                if si is not None and si.on_wait and len(si.on_wait) > 1:
                    waits = list(si.on_wait)
                    for w in waits[:-1]:
                        uid += 1
                        new_insts.append(
                            mybir.InstNoOp(
                                name=f"I-waitsplit-{uid}",
                                engine=inst.engine,
                                ins=[],
                                outs=[],
                                sync_info=mybir.SyncInfo(on_wait=[w], on_update=[]),
                            )
                        )
                    inst.sync_info = mybir.SyncInfo(
                        on_wait=[waits[-1]], on_update=si.on_update
                    )
                new_insts.append(inst)
            blk.instructions = new_insts
    return nc


def build_nc(
    in_dim=IN_DIM,
    per_out=PER_OUT,
    num_groups=NUM_GROUPS,
    groups_per_chunk=8,
    w_bufs=7,
    mixed_halves=None,  # {half_index: n_groups_diverted_E79-free}
    split_waits=True,
):
    if mixed_halves is None:
        mixed_halves = {6: 2, 12: 2}
    import concourse.bass as bass
    import concourse.mybir as mybir
    import concourse.tile as tile

    f32 = mybir.dt.float32
    bf16 = mybir.dt.bfloat16
    add = mybir.AluOpType.add
    mult = mybir.AluOpType.mult

    ng = num_groups
    gpc = groups_per_chunk
    n_chunks = ng // gpc
    oc_n = per_out // P  # out-chunks of 128
    hpc = 2  # halves per chunk
    gph = gpc // hpc  # groups per half (4)
    n_halves = n_chunks * hpc
    assert ng % gpc == 0 and per_out % P == 0 and in_dim == ng * GROUP_SIZE

    nc = bass.Bass()
    # host-permuted weights: wt[c, ch, gp, o] = W[o, ch*1024 + gp*128 + c]
    wt = nc.dram_tensor("wt", [P, n_chunks, gpc, per_out], f32, kind="ExternalInput")
    # host-transposed x: x[c, g] = x_full[g*128 + c]
    x_d = nc.dram_tensor("x", [P, ng], f32, kind="ExternalInput")
    # host-arranged scales, duplicated over the hi/lo axis:
    # sc[p, oc, g, j] = scales[oc*128 + p, g]
    sc_d = nc.dram_tensor("scales", [P, oc_n, ng, 2], f32, kind="ExternalInput")
    # out[p, oc] = result[oc*128 + p]; host un-permutes
    out_d = nc.dram_tensor("out", [P, oc_n], f32, kind="ExternalOutput")

    with tile.TileContext(nc) as tc:
        with (
            tc.tile_pool(name="singles", bufs=1) as singles,
            tc.tile_pool(name="w", bufs=w_bufs) as wpool,
            tc.tile_pool(name="wg", bufs=4) as wgpool,
            tc.tile_pool(name="q", bufs=2) as qpool,
            tc.tile_pool(name="ep", bufs=2) as epool,
            tc.tile_pool(name="psum", bufs=2, space="PSUM") as psum,
        ):
            xT = singles.tile([P, ng], f32)
            sc_sb = singles.tile([P, oc_n, ng, 2], f32)
            x2 = singles.tile([P, ng, 2], bf16)
            xhi32 = singles.tile([P, ng], f32)
            xlo32 = singles.tile([P, ng], f32)
            out_acc = singles.tile([P, oc_n], f32, name="oacc")

            # DVE warmup: touch every DVE op type once so the runtime's
            # dve-table loads (4 x 16 KiB on engine E64's queue) happen now,
            # during startup slack, instead of stalling E64 mid-stream.
            wa = singles.tile([P, 4], f32)
            wb = singles.tile([P, 4], f32)
            wc = singles.tile([P, 1], f32)
            nc.gpsimd.memset(wa[:, :], 0.0)
            nc.vector.tensor_scalar(
                out=wb, in0=wa, scalar1=0.0, scalar2=0.0, op0=add, op1=add
            )
            nc.vector.tensor_copy(out=wa, in_=wb)
            nc.vector.tensor_tensor(wb, wa, wa, mult)
            nc.vector.reduce_sum(out=wc, in_=wb, axis=mybir.AxisListType.X)

            # one persistent PSUM accumulator for the whole run (2 banks):
            # acc_all[:, oc, g, 2] with g = ch*gpc + gp
            acc_all = psum.tile([P, oc_n, ng, 2], f32, tag="accall")

            qw = None
            for h in range(n_halves):
                ch, hh = divmod(h, hpc)
                gp0 = hh * gph
                src = wt[:, ch, gp0 : gp0 + gph, :]

                if hh == 0:
                    qw = qpool.tile([P, gpc, per_out], bf16, tag="qw")

                # ---- weight DMAs for this half (sync ring)
                if h == n_halves - 1:
                    # final half: per-group DMAs so the tail quantize is tiny
                    parts = []
                    for gp in range(gph):
                        wf = wgpool.tile([P, 1, per_out], f32, tag="wg")
                        nc.sync.dma_start(wf, src[:, gp : gp + 1, :])
                        parts.append((wf, gp0 + gp, 1))
                elif h in mixed_halves:
                    # mixed: uniform part + E79-free part + remainder
                    wf = wpool.tile([P, gph, per_out], f32, tag="wf")
                    ug = gph - mixed_halves[h]
                    if ug:
                        nc.sync.dma_start(wf[:, 0:ug, :], src[:, 0:ug, :])
                    nc.sync.dma_start(wf[0:120, ug:gph, :], src[0:120, ug:gph, :])
                    nc.sync.dma_start(
                        wf[120:128, ug:gph, :],
                        src[120:128, ug:gph, :],
                        max_dma_last_dim=1024,
                    )
                    parts = [(wf, gp0, gph)]
                else:
                    # uniform: one full half-tile DMA, 16 KiB descriptors
                    wf = wpool.tile([P, gph, per_out], f32, tag="wf")
                    nc.sync.dma_start(wf, src)
                    parts = [(wf, gp0, gph)]

                if h == 0:
                    # tiny x/scales loads on the scalar ring, in parallel
                    nc.scalar.dma_start(xT, x_d[:, :])
                    nc.scalar.dma_start(sc_sb, sc_d[:, :, :, :])
                    # Dekker split: x2 = [bf16(x) | bf16(x - hi)]
                    nc.vector.tensor_copy(out=x2[:, :, 0], in_=xT)
                    nc.vector.tensor_copy(out=xhi32, in_=x2[:, :, 0])
                    nc.vector.tensor_tensor(
                        xlo32, xT, xhi32, mybir.AluOpType.subtract
                    )
                    nc.vector.tensor_copy(out=x2[:, :, 1], in_=xlo32)

                # ---- quantize + matmuls per loaded part
                for wf, pgp0, pgpn in parts:
                    nc.vector.tensor_scalar(
                        out=qw[:, pgp0 : pgp0 + pgpn, :],
                        in0=wf,
                        scalar1=float(MAGIC), scalar2=-float(MAGIC),
                        op0=add, op1=add,
                    )
                    for k in range(pgpn):
                        gp = pgp0 + k
                        g = ch * gpc + gp
                        for oc in range(oc_n):
                            nc.tensor.matmul(
                                acc_all[:, oc, g, :],
                                lhsT=qw[:, gp, oc * P : (oc + 1) * P],
                                rhs=x2[:, g, :],
                                start=True,
                                stop=True,
                            )

            # ---- single-shot epilogue: out[p, oc] = sum_{g,j} acc * scales
            y = epool.tile([P, oc_n, ng, 2], f32, tag="y")
            nc.vector.tensor_tensor(y, acc_all[:, :, :, :], sc_sb[:, :, :, :], mult)
            nc.vector.reduce_sum(
                out=out_acc, in_=y, axis=mybir.AxisListType.XY
            )
            nc.scalar.dma_start(out_d[:, :], out_acc)

    return _split_multi_waits(nc) if split_waits else nc


def _prep_in_maps(x, weights, scales):
    """Pure-layout host prep: shard + permute per core."""
    x = np.asarray(x, dtype=np.float32)
    weights = np.asarray(weights, dtype=np.float32)
    scales = np.asarray(scales, dtype=np.float32)

    gpc = 8
    n_chunks = NUM_GROUPS // gpc
    oc_n = PER_OUT // P

    xT = np.ascontiguousarray(x.reshape(NUM_GROUPS, P).T)  # [128, 64]
    in_maps = []
    for c in range(N_CORES):
        sl = slice(c * PER_OUT, (c + 1) * PER_OUT)
        w_sl = weights[sl]  # [1024, 8192]
        # wt[c, ch, gp, o] = W[o, ch*1024 + gp*128 + c]
        wt = np.ascontiguousarray(
            w_sl.reshape(PER_OUT, n_chunks, gpc, P).transpose(3, 1, 2, 0)
        )
        s_sl = scales[sl]  # [1024, 64]
        # sc[p, oc, g, j] = scales[oc*128 + p, g]
        sc = s_sl.reshape(oc_n, P, NUM_GROUPS).transpose(1, 0, 2)
        sc2 = np.ascontiguousarray(
            np.broadcast_to(sc[..., None], (P, oc_n, NUM_GROUPS, 2))
        )
        in_maps.append({"wt": wt, "x": xT, "scales": sc2})
    return in_maps


def kernel(x, weights, scales):
    from concourse import bass_utils

    if "nc" not in _cache:
        _cache["nc"] = build_nc()
    nc = _cache["nc"]

    in_maps = _prep_in_maps(x, weights, scales)
    res = bass_utils.run_bass_kernel_spmd(nc, in_maps, core_ids=list(range(N_CORES)))
    # out[p, oc] -> result[oc*128 + p]
    return np.concatenate(
        [res.results[c]["out"].T.reshape(-1) for c in range(N_CORES)]
    ).astype(np.float32)
